# revision 3
# baseline (speedup 1.0000x reference)
"""Multi-head attention block (qkv -> attention -> o_net -> residual+LN) on
8 Trainium2 NeuronCores.

Problem (hardcoded): B=2, T=2048, D=1024, H=16, dh=64, fp32 I/O.
Reference quirk: the (B,H,T,dh) attention buffer is viewed as (H,B,T,dh)
before the output projection, i.e. output batch b2 / head-slot h2 takes the
attention output of original (b, h) with 16*b + h == 2*h2 + b2.

Sharding: tokens split along T only -> core c owns positions
[c*256, (c+1)*256) of BOTH batches (512 tokens).  Every core computes qkv for
its tokens, AllGathers K and V (Q stays local), runs attention for ALL 32
(b,h) pairs restricted to its query positions, applies the o_net with the
permutation above, then residual + layernorm on its tokens.  All device
addresses are identical across cores (pure SPMD); only the data differs.
"""
import sys
sys.path.insert(0, "/opt/trn_rl_repo")
import contextlib
import numpy as np
import ml_dtypes

import concourse.bass as bass
from concourse import bacc
import concourse.mybir as mybir
import concourse.tile as tile
from concourse.bass_utils import run_bass_kernel_spmd

BF16 = mybir.dt.bfloat16
F32 = mybir.dt.float32
nbf16 = ml_dtypes.bfloat16

N_CORES = 8
B, T, D = 2, 2048, 1024
H, DH = 16, 64
TC = T // N_CORES          # 256 query positions per core
NTOK = B * TC              # 512 tokens per core (both batches)
LN_EPS = 1e-5

KT_ROWS = 1024             # k-channel rows in the AG buffer
V_W = H * (DH + 1)         # 1040: V row width with ones column per head
KT_SZ = KT_ROWS * NTOK     # 524288 elems
V_SZ = NTOK * V_W          # 532480 elems
AGS = KT_SZ + V_SZ         # per-rank AG elems (1,056,768)

_prog_cache = {}


def _build_program():
    nc = bacc.Bacc("TRN2", num_devices=N_CORES)

    # ---- per-core inputs (host pre-tiled / pre-transposed, bf16) ----
    inpT = nc.dram_tensor("inpT", [128, 8, NTOK], BF16, kind="ExternalInput")
    inp_res = nc.dram_tensor("inp_res", [NTOK, D], F32, kind="ExternalInput")
    wqkT = nc.dram_tensor("wqkT", [128, 8, 2048], BF16, kind="ExternalInput")
    wvT = nc.dram_tensor("wvT", [128, 8, 1024], BF16, kind="ExternalInput")
    woT = nc.dram_tensor("woT", [128, 8, 1024], BF16, kind="ExternalInput")
    b_qk = nc.dram_tensor("b_qk", [1, 2048], BF16, kind="ExternalInput")
    b_v = nc.dram_tensor("b_v", [1, 1024], BF16, kind="ExternalInput")
    onesd = nc.dram_tensor("onesd", [1, NTOK], BF16, kind="ExternalInput")
    gamma = nc.dram_tensor("gamma", [1, D], F32, kind="ExternalInput")
    beta = nc.dram_tensor("beta", [1, D], F32, kind="ExternalInput")

    out = nc.dram_tensor("out", [NTOK, D], F32, kind="ExternalOutput")

    def bcast_rows(src_row_ap, nrows):
        return bass.AP(tensor=src_row_ap.tensor, offset=src_row_ap.offset,
                       ap=[[0, nrows]] + src_row_ap.ap[1:])

    with tile.TileContext(nc) as tc:
        with contextlib.ExitStack() as ctx:
            dram = ctx.enter_context(tc.tile_pool(name="dram", bufs=1, space="DRAM"))
            dram_sc = ctx.enter_context(tc.tile_pool(name="dram_sc", bufs=4, space="DRAM"))
            cst = ctx.enter_context(tc.tile_pool(name="cst", bufs=1))

            agin = dram.tile([AGS], BF16)
            agout = dram.tile([N_CORES, AGS], BF16, addr_space="Shared")

            ones_sb = cst.tile([1, NTOK], BF16)
            nc.sync.dma_start(out=ones_sb[:], in_=onesd[:])
            bqk_sb = cst.tile([1, 2048], BF16)
            nc.sync.dma_start(out=bqk_sb[:], in_=b_qk[:])
            bv_sb = cst.tile([1, 1024], BF16)
            nc.sync.dma_start(out=bv_sb[:], in_=b_v[:])

            qT_sb = cst.tile([128, 8, NTOK], BF16)   # Q^T kept on-chip

            # ---------------- phase 1: qkv projection -----------------
            with tc.tile_pool(name="projw", bufs=1) as projw, \
                 tc.tile_pool(name="pstage", bufs=3) as pstage, \
                 tc.tile_pool(name="psproj", bufs=4, space="PSUM") as psproj:
                wqk_sb = projw.tile([128, 8, 2048], BF16)
                nc.sync.dma_start(out=wqk_sb[:], in_=wqkT[:])
                wv_sb = projw.tile([128, 8, 1024], BF16)
                nc.sync.dma_start(out=wv_sb[:], in_=wvT[:])
                inpT_sb = projw.tile([128, 8, NTOK], BF16)
                nc.sync.dma_start(out=inpT_sb[:], in_=inpT[:])

                # --- K^T: channels 1024..2047 of qkv, layout [1024, 512] ---
                for cc in range(8):
                    pp = psproj.tile([128, NTOK], F32, tag="pp")
                    nc.tensor.matmul(out=pp[:], lhsT=bqk_sb[0:1, 1024 + cc * 128: 1024 + (cc + 1) * 128],
                                     rhs=ones_sb[:], start=True, stop=False)
                    for dt in range(8):
                        nc.tensor.matmul(out=pp[:], lhsT=wqk_sb[:, dt, 1024 + cc * 128: 1024 + (cc + 1) * 128],
                                         rhs=inpT_sb[:, dt, :], start=False, stop=(dt == 7))
                    ks = pstage.tile([128, NTOK], BF16, tag="ks")
                    nc.vector.tensor_copy(out=ks[:], in_=pp[:])
                    dst = agin[cc * 128 * NTOK: (cc + 1) * 128 * NTOK]
                    nc.sync.dma_start(out=dst.rearrange("(p f) -> p f", p=128), in_=ks[:])

                # --- V: [512 tok, 1040] with ones col per head ---
                for tch in range(4):
                    vs = pstage.tile([128, H, DH + 1], BF16, tag="vs")
                    nc.vector.memset(vs[:, :, DH: DH + 1], 1.0)
                    for ncv in range(2):
                        pv = psproj.tile([128, 512], F32, tag="pv")
                        nc.tensor.matmul(out=pv[:], lhsT=ones_sb[0:1, 0:128],
                                         rhs=bv_sb[0:1, ncv * 512: (ncv + 1) * 512],
                                         start=True, stop=False)
                        for dt in range(8):
                            nc.tensor.matmul(out=pv[:], lhsT=inpT_sb[:, dt, tch * 128: (tch + 1) * 128],
                                             rhs=wv_sb[:, dt, ncv * 512: (ncv + 1) * 512],
                                             start=False, stop=(dt == 7))
                        nc.vector.tensor_copy(
                            out=vs[:, ncv * 8: (ncv + 1) * 8, 0:DH],
                            in_=pv[:].rearrange("p (h d) -> p h d", d=DH))
                    dst = agin[KT_SZ + tch * 128 * V_W: KT_SZ + (tch + 1) * 128 * V_W]
                    nc.sync.dma_start(out=dst.rearrange("(p f) -> p f", p=128), in_=vs[:])

                # --- AllGather of K^T + V' ---
                nc.gpsimd.collective_compute(
                    "AllGather", mybir.AluOpType.bypass,
                    replica_groups=[list(range(N_CORES))],
                    ins=[agin[:]], outs=[agout[:]],
                )

                # --- Q^T: channels 0..1023, kept in SBUF (overlaps AG) ---
                for cc in range(8):
                    pp = psproj.tile([128, NTOK], F32, tag="pp")
                    nc.tensor.matmul(out=pp[:], lhsT=bqk_sb[0:1, cc * 128: (cc + 1) * 128],
                                     rhs=ones_sb[:], start=True, stop=False)
                    for dt in range(8):
                        nc.tensor.matmul(out=pp[:], lhsT=wqk_sb[:, dt, cc * 128: (cc + 1) * 128],
                                         rhs=inpT_sb[:, dt, :], start=False, stop=(dt == 7))
                    nc.vector.tensor_copy(out=qT_sb[:, cc, :], in_=pp[:])

            # ---------------- phase 3: attention over 16 duos ----------
            # duo d: pairs (2d, 2d+1); b' = d//8, h'_A = 2d%16 (even), h'_B = h'_A+1
            # output: duo d -> slot d; pair A -> out batch 0, pair B -> batch 1
            attnv = [cst.tile([128, 8, TC], BF16, name=f"attnv{b2}") for b2 in range(2)]

            with tc.tile_pool(name="vfull", bufs=2) as vfull_pool, \
                 tc.tile_pool(name="att", bufs=2) as att, \
                 tc.tile_pool(name="pt", bufs=3) as ptp, \
                 tc.tile_pool(name="nrm", bufs=4) as nrm, \
                 tc.tile_pool(name="pss", bufs=3, space="PSUM") as pss, \
                 tc.tile_pool(name="pso", bufs=1, space="PSUM") as pso:

                vfull = None
                for d in range(16):
                    bp = d // 8
                    hA = (2 * d) % 16
                    row_off = 128 * (d % 8)

                    if d % 8 == 0:
                        # load V' for batch bp: [128, kt, head, 65]
                        vfull = vfull_pool.tile([128, 16, H, DH + 1], BF16, tag="vf")
                        for kt in range(16):
                            r = kt // 2
                            off = r * AGS + KT_SZ + (bp * TC + (kt % 2) * 128) * V_W
                            src = bass.AP(tensor=agout.tensor, offset=agout.offset + off,
                                          ap=[[V_W, 128], [1, V_W]])
                            nc.sync.dma_start(
                                out=vfull[:, kt, :, :].rearrange("p h d -> p (h d)"), in_=src)

                    # K^T duo [128 rows, 8 ranks, 256]
                    ktd = att.tile([128, 8, TC], BF16, tag="ktd")
                    src = bass.AP(tensor=agout.tensor,
                                  offset=agout.offset + row_off * NTOK + bp * TC,
                                  ap=[[NTOK, 128], [AGS, 8], [1, TC]])
                    nc.sync.dma_start(out=ktd[:], in_=src)

                    qd = qT_sb[:, d % 8, bp * TC: (bp + 1) * TC]   # [128, 256]

                    poA = pso.tile([65, TC], F32, tag="poA")
                    poB = pso.tile([65, TC], F32, tag="poB")
                    for blk in range(8):
                        pscr = pss.tile([128, 1024], F32, tag="pscr")
                        for j2 in range(2):
                            kt = blk * 2 + j2
                            nc.tensor.matmul(out=pscr[:, j2 * TC: (j2 + 1) * TC],
                                             lhsT=ktd[0:64, kt // 2, (kt % 2) * 128: (kt % 2 + 1) * 128],
                                             rhs=qd[0:64, :],
                                             start=True, stop=True, tile_position=(0, 0))
                            nc.tensor.matmul(out=pscr[:, 512 + j2 * TC: 512 + (j2 + 1) * TC],
                                             lhsT=ktd[64:128, kt // 2, (kt % 2) * 128: (kt % 2 + 1) * 128],
                                             rhs=qd[64:128, :],
                                             start=True, stop=True, tile_position=(64, 0))
                        pt = ptp.tile([128, 1024], BF16, tag="pt")
                        nc.scalar.activation(out=pt[:], in_=pscr[:],
                                             func=mybir.ActivationFunctionType.Exp, scale=0.125)
                        for j2 in range(2):
                            kt = blk * 2 + j2
                            nc.tensor.matmul(out=poA[:], lhsT=vfull[:, kt, hA, :],
                                             rhs=pt[:, j2 * TC: (j2 + 1) * TC],
                                             start=(kt == 0), stop=(kt == 15))
                            nc.tensor.matmul(out=poB[:], lhsT=vfull[:, kt, hA + 1, :],
                                             rhs=pt[:, 512 + j2 * TC: 512 + (j2 + 1) * TC],
                                             start=(kt == 0), stop=(kt == 15))

                    # normalize: rec = 1/denominator, broadcast via DRAM bounce
                    recA = nrm.tile([1, TC], F32, tag="recA")
                    nc.vector.reciprocal(out=recA[:], in_=poA[64:65, :])
                    recB = nrm.tile([1, TC], F32, tag="recB")
                    nc.vector.reciprocal(out=recB[:], in_=poB[64:65, :])
                    rec_d = dram_sc.tile([2, TC], F32, tag="rec_d")
                    nc.sync.dma_start(out=rec_d[0:1, :], in_=recA[:])
                    nc.sync.dma_start(out=rec_d[1:2, :], in_=recB[:])
                    rb = nrm.tile([128, TC], F32, tag="rb")
                    for h2 in range(2):
                        srcr = rec_d[h2: h2 + 1, :]
                        nc.gpsimd.dma_start(out=rb[64 * h2: 64 * h2 + 64, :],
                                            in_=bcast_rows(srcr, 64))
                    jt, rhalf = d // 2, (d % 2) * 64
                    nc.vector.tensor_tensor(out=attnv[0][rhalf: rhalf + 64, jt, :],
                                            in0=poA[0:64, :], in1=rb[0:64, :],
                                            op=mybir.AluOpType.mult)
                    nc.vector.tensor_tensor(out=attnv[1][rhalf: rhalf + 64, jt, :],
                                            in0=poB[0:64, :], in1=rb[64:128, :],
                                            op=mybir.AluOpType.mult)

            # ---------------- phase 4: o_net + residual + layernorm ----
            with tc.tile_pool(name="fin", bufs=2) as fin, \
                 tc.tile_pool(name="finc", bufs=1) as finc, \
                 tc.tile_pool(name="psf", bufs=4, space="PSUM") as psf:
                wo_sb = finc.tile([128, 8, 1024], BF16)
                nc.sync.dma_start(out=wo_sb[:], in_=woT[:])
                res_sb = finc.tile([128, 4, D], F32)
                nc.sync.dma_start(out=res_sb[:],
                                  in_=inp_res.rearrange("(c p) d -> p c d", p=128))
                gb_sb = finc.tile([128, D], F32)
                nc.gpsimd.dma_start(out=gb_sb[:], in_=bcast_rows(gamma[0:1, :], 128))
                bb_sb = finc.tile([128, D], F32)
                nc.gpsimd.dma_start(out=bb_sb[:], in_=bcast_rows(beta[0:1, :], 128))
                eps_sb = finc.tile([128, 1], F32)
                nc.vector.memset(eps_sb[:], LN_EPS)

                for b2 in range(2):
                    for tch in range(2):
                        chunk = b2 * 2 + tch
                        x = fin.tile([128, D], F32, tag="x")
                        for nn_ in range(2):
                            po = psf.tile([128, 512], F32, tag="po")
                            for jt in range(8):
                                nc.tensor.matmul(out=po[:],
                                                 lhsT=attnv[b2][:, jt, tch * 128: (tch + 1) * 128],
                                                 rhs=wo_sb[:, jt, nn_ * 512: (nn_ + 1) * 512],
                                                 start=(jt == 0), stop=(jt == 7))
                            nc.vector.tensor_tensor(out=x[:, nn_ * 512: (nn_ + 1) * 512],
                                                    in0=po[:],
                                                    in1=res_sb[:, chunk, nn_ * 512: (nn_ + 1) * 512],
                                                    op=mybir.AluOpType.add)
                        stats = fin.tile([128, 2, 6], F32, tag="stats")
                        for s2 in range(2):
                            nc.vector.bn_stats(out=stats[:, s2, :], in_=x[:, s2 * 512: (s2 + 1) * 512])
                        mv = fin.tile([128, 2], F32, tag="mv")
                        nc.vector.bn_aggr(out=mv[:], in_=stats[:])
                        sd = fin.tile([128, 1], F32, tag="sd")
                        nc.scalar.activation(out=sd[:], in_=mv[:, 1:2],
                                             func=mybir.ActivationFunctionType.Sqrt,
                                             bias=eps_sb[:], scale=1.0)
                        rstd = fin.tile([128, 1], F32, tag="rstd")
                        nc.vector.reciprocal(out=rstd[:], in_=sd[:])
                        y = fin.tile([128, D], F32, tag="y")
                        nc.vector.tensor_scalar(out=y[:], in0=x[:],
                                                scalar1=mv[:, 0:1], scalar2=rstd[:],
                                                op0=mybir.AluOpType.subtract,
                                                op1=mybir.AluOpType.mult)
                        yg = fin.tile([128, D], F32, tag="yg")
                        nc.vector.tensor_tensor(out=yg[:], in0=y[:], in1=gb_sb[:],
                                                op=mybir.AluOpType.mult)
                        yb = fin.tile([128, D], F32, tag="yb")
                        nc.vector.tensor_tensor(out=yb[:], in0=yg[:], in1=bb_sb[:],
                                                op=mybir.AluOpType.add)
                        nc.sync.dma_start(
                            out=out[chunk * 128: (chunk + 1) * 128, :], in_=yb[:])

    nc.finalize()
    return nc


def _get_program():
    if "nc" not in _prog_cache:
        _prog_cache["nc"] = _build_program()
    return _prog_cache["nc"]


def _prep_inputs(inp, W_qkv, b_qkv, W_o, gamma, beta):
    """Build the 8 per-core input dicts (host-side, all free)."""
    f32 = np.float32
    inp = np.asarray(inp, f32)
    W_qkv = np.asarray(W_qkv, f32)
    b_qkv = np.asarray(b_qkv, f32)
    W_o = np.asarray(W_o, f32)
    gamma = np.asarray(gamma, f32).reshape(1, D)
    beta = np.asarray(beta, f32).reshape(1, D)

    wqkT = np.ascontiguousarray(
        W_qkv[0:2048, :].T.reshape(8, 128, 2048).transpose(1, 0, 2)).astype(nbf16)
    wvT = np.ascontiguousarray(
        W_qkv[2048:3072, :].T.reshape(8, 128, 1024).transpose(1, 0, 2)).astype(nbf16)
    woT = np.ascontiguousarray(
        W_o.T.reshape(8, 128, 1024).transpose(1, 0, 2)).astype(nbf16)
    b_qk = b_qkv[0:2048].reshape(1, 2048).astype(nbf16)
    b_v = b_qkv[2048:3072].reshape(1, 1024).astype(nbf16)
    ones = np.ones((1, NTOK), nbf16)

    in_maps = []
    for c in range(N_CORES):
        sl = slice(c * TC, (c + 1) * TC)
        x = np.concatenate([inp[0, sl, :], inp[1, sl, :]], axis=0)  # [512, 1024]
        inpT = np.ascontiguousarray(
            x.T.reshape(8, 128, NTOK).transpose(1, 0, 2)).astype(nbf16)
        in_maps.append({
            "inpT": inpT,
            "inp_res": np.ascontiguousarray(x),
            "wqkT": wqkT, "wvT": wvT, "woT": woT,
            "b_qk": b_qk, "b_v": b_v, "onesd": ones,
            "gamma": gamma, "beta": beta,
        })
    return in_maps


def _assemble(results):
    out = np.empty((B, T, D), np.float32)
    for c in range(N_CORES):
        o = results[c]["out"]
        sl = slice(c * TC, (c + 1) * TC)
        out[0, sl, :] = o[0:TC, :]
        out[1, sl, :] = o[TC:NTOK, :]
    return out


def kernel(inp, W_qkv, b_qkv, W_o, gamma, beta):
    nc = _get_program()
    in_maps = _prep_inputs(inp, W_qkv, b_qkv, W_o, gamma, beta)
    res = run_bass_kernel_spmd(nc, in_maps, core_ids=list(range(N_CORES)))
    return _assemble(res.results)


if __name__ == "__main__":
    rng = np.random.RandomState(0)
    inp = rng.randn(B, T, D).astype(np.float32)
    W_qkv = (rng.randn(3 * H * DH, D) * D ** -0.5).astype(np.float32)
    b_qkv = (rng.randn(3 * H * DH) * 0.02).astype(np.float32)
    W_o = (rng.randn(D, H * DH) * (H * DH) ** -0.5).astype(np.float32)
    gamma = np.ones(D, np.float32)
    beta = np.zeros(D, np.float32)
    out = kernel(inp=inp, W_qkv=W_qkv, b_qkv=b_qkv, W_o=W_o, gamma=gamma, beta=beta)
    print("out", out.shape, out.dtype, np.abs(out).mean())


# revision 30
# speedup vs baseline: 6.2836x; 6.2836x over previous
"""Multi-head attention block (qkv -> attention -> o_net -> residual+LN) on
8 Trainium2 NeuronCores.

Problem (hardcoded): B=2, T=2048, D=1024, H=16, dh=64, fp32 I/O.
Reference quirk: the (B,H,T,dh) attention buffer is viewed as (H,B,T,dh)
before the output projection, i.e. output batch b2 / head-slot h2 takes the
attention output of original (b, h) with 16*b + h == 2*h2 + b2.

Sharding: tokens split along T only -> core c owns positions
[c*256, (c+1)*256) of BOTH batches (512 tokens).  Every core computes qkv for
its tokens, AllGathers K and V (Q stays local), runs attention for ALL 32
(b,h) pairs restricted to its query positions, applies the o_net with the
permutation above, then residual + layernorm on its tokens.  All device
addresses are identical across cores (pure SPMD); only the data differs.
"""
import sys
sys.path.insert(0, "/opt/trn_rl_repo")
import contextlib
import numpy as np
import ml_dtypes

import concourse.bass as bass
from concourse import bacc
import concourse.mybir as mybir
import concourse.tile as tile
from concourse.bass_utils import run_bass_kernel_spmd

BF16 = mybir.dt.bfloat16
F32 = mybir.dt.float32
nbf16 = ml_dtypes.bfloat16

N_CORES = 8
B, T, D = 2, 2048, 1024
H, DH = 16, 64
TC = T // N_CORES          # 256 query positions per core
NTOK = B * TC              # 512 tokens per core (both batches)
LN_EPS = 1e-5

KT_ROWS = 1024             # k-channel rows in the AG buffer
V_W = H * (DH + 1)         # 1040: V row width with ones column per head
KT_SZ = KT_ROWS * NTOK     # 524288 elems
V_SZ = NTOK * V_W          # 532480 elems
AGS = KT_SZ + V_SZ         # per-rank AG elems (1,056,768)

_prog_cache = {}

# bf16 scores in PSUM: halves the exp instruction count (bigger ACT ops).
SCORE_BF16 = False


def _build_program(reps=1, score_bf16=SCORE_BF16):
    """reps>1 repeats the attention + o_net phases (timing-only builds)."""
    nc = bacc.Bacc("TRN2", num_devices=N_CORES)

    # ---- per-core inputs (host pre-tiled / pre-transposed, bf16) ----
    inpT = nc.dram_tensor("inpT", [128, 8, NTOK], BF16, kind="ExternalInput")
    inp_res = nc.dram_tensor("inp_res", [NTOK, D], F32, kind="ExternalInput")
    wqkT = nc.dram_tensor("wqkT", [128, 8, 2048], BF16, kind="ExternalInput")
    wvT = nc.dram_tensor("wvT", [128, 8, 1024], BF16, kind="ExternalInput")
    woT = nc.dram_tensor("woT", [128, 8, 1024], BF16, kind="ExternalInput")
    b_qk = nc.dram_tensor("b_qk", [1, 2048], BF16, kind="ExternalInput")
    b_v = nc.dram_tensor("b_v", [1, 1024], BF16, kind="ExternalInput")
    onesd = nc.dram_tensor("onesd", [1, NTOK], BF16, kind="ExternalInput")
    gamma = nc.dram_tensor("gamma", [1, D], F32, kind="ExternalInput")
    beta = nc.dram_tensor("beta", [1, D], F32, kind="ExternalInput")

    out = nc.dram_tensor("out", [NTOK, D], F32, kind="ExternalOutput")

    def bcast_rows(src_row_ap, nrows):
        return bass.AP(tensor=src_row_ap.tensor, offset=src_row_ap.offset,
                       ap=[[0, nrows]] + src_row_ap.ap[1:])

    # scores-block layout: pair A fills the first half of the tile's banks,
    # pair B the second (concurrent row-packed matmuls must target different
    # PSUM banks, so each pair-half must be >= 1 full bank -> block >= 2 kt).
    # Empirically fastest: 8 blocks of 2 kt, double-buffered:
    #   pscr2 (2 banks x 2 bufs) + poA (1) + poB (1) = 6 of 8 banks.
    import os as _os
    _blk = _os.environ.get("KBLOCKS", "2,2,2,2,2,2,2,2")
    SCORE_DT, BLOCKS = F32, [int(x) for x in _blk.split(",")]
    assert sum(BLOCKS) == 16
    PSB = int(_os.environ.get("KPSB", "2"))
    EVAC = _os.environ.get("KEVAC", "1") == "1"
    PROBE = _os.environ.get("KPROBE", "")  # timing-only ablations

    with tile.TileContext(nc) as tc:
        with contextlib.ExitStack() as ctx:
            dram = ctx.enter_context(tc.tile_pool(name="dram", bufs=1, space="DRAM"))
            dram_sc = ctx.enter_context(tc.tile_pool(name="dram_sc", bufs=4, space="DRAM"))
            cst = ctx.enter_context(tc.tile_pool(name="cst", bufs=1))

            agin = dram.tile([AGS], BF16)
            agout_k = dram.tile([N_CORES, KT_SZ], BF16, addr_space="Shared")
            agout_v = dram.tile([N_CORES, V_SZ], BF16, addr_space="Shared")

            ones_sb = cst.tile([1, NTOK], BF16)
            nc.sync.dma_start(out=ones_sb[:], in_=onesd[:])
            bqk_sb = cst.tile([1, 2048], BF16)
            nc.sync.dma_start(out=bqk_sb[:], in_=b_qk[:])
            bv_sb = cst.tile([1, 1024], BF16)
            nc.sync.dma_start(out=bv_sb[:], in_=b_v[:])

            qT_sb = cst.tile([128, 8, NTOK], BF16)   # Q^T kept on-chip

            # ---------------- phase 1: qkv projection -----------------
            with tc.tile_pool(name="projw", bufs=1) as projw, \
                 tc.tile_pool(name="pstage", bufs=3) as pstage, \
                 tc.tile_pool(name="psproj", bufs=4, space="PSUM") as psproj:
                wqk_sb = projw.tile([128, 8, 2048], BF16)
                wv_sb = projw.tile([128, 8, 1024], BF16)
                inpT_sb = projw.tile([128, 8, NTOK], BF16)
                for dt in range(8):
                    nc.sync.dma_start(out=inpT_sb[:, dt, :], in_=inpT[:, dt, :])
                    nc.sync.dma_start(out=wqk_sb[:, dt, :], in_=wqkT[:, dt, :])
                    nc.sync.dma_start(out=wv_sb[:, dt, :], in_=wvT[:, dt, :])

                # --- K^T: channels 1024..2047 of qkv, layout [1024, 512] ---
                for cc in range(8):
                    pp = psproj.tile([128, NTOK], F32, tag="pp")
                    nc.tensor.matmul(out=pp[:], lhsT=bqk_sb[0:1, 1024 + cc * 128: 1024 + (cc + 1) * 128],
                                     rhs=ones_sb[:], start=True, stop=False)
                    for dt in range(8):
                        nc.tensor.matmul(out=pp[:], lhsT=wqk_sb[:, dt, 1024 + cc * 128: 1024 + (cc + 1) * 128],
                                         rhs=inpT_sb[:, dt, :], start=False, stop=(dt == 7))
                    ks = pstage.tile([128, NTOK], BF16, tag="ks")
                    nc.vector.tensor_copy(out=ks[:], in_=pp[:])
                    dst = agin[cc * 128 * NTOK: (cc + 1) * 128 * NTOK]
                    nc.sync.dma_start(out=dst.rearrange("(p f) -> p f", p=128), in_=ks[:])

                # --- AllGather of K^T as soon as it's staged ---
                nc.gpsimd.collective_compute(
                    "AllGather", mybir.AluOpType.bypass,
                    replica_groups=[list(range(N_CORES))],
                    ins=[agin[0:KT_SZ]], outs=[agout_k[:]],
                )


                # --- Q^T: channels 0..1023, kept in SBUF (overlaps AG) ---
                for cc in range(8):
                    pp = psproj.tile([128, NTOK], F32, tag="pp")
                    nc.tensor.matmul(out=pp[:], lhsT=bqk_sb[0:1, cc * 128: (cc + 1) * 128],
                                     rhs=ones_sb[:], start=True, stop=False)
                    for dt in range(8):
                        nc.tensor.matmul(out=pp[:], lhsT=wqk_sb[:, dt, cc * 128: (cc + 1) * 128],
                                         rhs=inpT_sb[:, dt, :], start=False, stop=(dt == 7))
                    nc.vector.tensor_copy(out=qT_sb[:, cc, :], in_=pp[:])

                # --- V: [512 tok, 1040] with ones col per head ---
                for tch in range(4):
                    vs = pstage.tile([128, H, DH + 1], BF16, tag="vs")
                    nc.vector.memset(vs[:, :, DH: DH + 1], 1.0)
                    for ncv in range(2):
                        pv = psproj.tile([128, 512], F32, tag="pv")
                        nc.tensor.matmul(out=pv[:], lhsT=ones_sb[0:1, 0:128],
                                         rhs=bv_sb[0:1, ncv * 512: (ncv + 1) * 512],
                                         start=True, stop=False)
                        for dt in range(8):
                            nc.tensor.matmul(out=pv[:], lhsT=inpT_sb[:, dt, tch * 128: (tch + 1) * 128],
                                             rhs=wv_sb[:, dt, ncv * 512: (ncv + 1) * 512],
                                             start=False, stop=(dt == 7))
                        nc.vector.tensor_copy(
                            out=vs[:, ncv * 8: (ncv + 1) * 8, 0:DH],
                            in_=pv[:].rearrange("p (h d) -> p h d", d=DH))
                    dst = agin[KT_SZ + tch * 128 * V_W: KT_SZ + (tch + 1) * 128 * V_W]
                    nc.sync.dma_start(out=dst.rearrange("(p f) -> p f", p=128), in_=vs[:])

                # --- AllGather of V' (K's AG was kicked right after K-proj) ---
                nc.gpsimd.collective_compute(
                    "AllGather", mybir.AluOpType.bypass,
                    replica_groups=[list(range(N_CORES))],
                    ins=[agin[KT_SZ:]], outs=[agout_v[:]],
                )

            # ---------------- phase 3: attention over 16 duos ----------
            # duo d: pairs (2d, 2d+1); b' = d//8, h'_A = 2d%16 (even), h'_B = h'_A+1
            # output: duo d -> slot d; pair A -> out batch 0, pair B -> batch 1
            attnv = [cst.tile([128, 8, TC], BF16, name=f"attnv{b2}") for b2 in range(2)]
            if reps == 0:   # timing-only build: keep o_net inputs defined
                nc.vector.memset(attnv[0][:], 0.0)
                nc.vector.memset(attnv[1][:], 0.0)

            for _rep in range(reps):
              with tc.tile_pool(name="vfull", bufs=2) as vfull_pool, \
                 tc.tile_pool(name="att", bufs=2) as att, \
                 tc.tile_pool(name="pt", bufs=3) as ptp, \
                 tc.tile_pool(name="nrm", bufs=3) as nrm, \
                 tc.tile_pool(name="pss", bufs=3, space="PSUM") as pss, \
                 tc.tile_pool(name="pso", bufs=1, space="PSUM") as pso:

                vfull = None
                for d in range(16):
                    bp = d // 8
                    hA = (2 * d) % 16
                    row_off = 128 * (d % 8)

                    if d % 8 == 0:
                        # load V' for batch bp: [128, kt, head, 65]
                        vfull = vfull_pool.tile([128, 16, H, DH + 1], BF16, tag="vf")
                        for kt in range(16):
                            r = kt // 2
                            off = r * V_SZ + (bp * TC + (kt % 2) * 128) * V_W
                            src = bass.AP(tensor=agout_v.tensor, offset=agout_v.offset + off,
                                          ap=[[V_W, 128], [1, V_W]])
                            nc.sync.dma_start(
                                out=vfull[:, kt, :, :].rearrange("p h d -> p (h d)"), in_=src)

                    # K^T duo [128 rows, 8 ranks, 256]
                    ktd = att.tile([128, 8, TC], BF16, tag="ktd")
                    src = bass.AP(tensor=agout_k.tensor,
                                  offset=agout_k.offset + row_off * NTOK + bp * TC,
                                  ap=[[NTOK, 128], [KT_SZ, 8], [1, TC]])
                    nc.sync.dma_start(out=ktd[:], in_=src)

                    qd = qT_sb[:, d % 8, bp * TC: (bp + 1) * TC]   # [128, 256]

                    poA = pso.tile([65, TC], F32, tag="poA")
                    poB = pso.tile([65, TC], F32, tag="poB")
                    kt0 = 0
                    for bw in BLOCKS:   # kt-tiles per scores block
                        half = bw * TC  # elems per pair-half (bank-aligned)
                        pscr = pss.tile([128, 2 * half], SCORE_DT,
                                        tag=f"pscr{bw}", bufs=PSB)
                        for j2 in range(bw):
                            kt = kt0 + j2
                            nc.tensor.matmul(out=pscr[:, j2 * TC: (j2 + 1) * TC],
                                             lhsT=ktd[0:64, kt // 2, (kt % 2) * 128: (kt % 2 + 1) * 128],
                                             rhs=qd[0:64, :],
                                             start=True, stop=True, tile_position=(0, 0))
                            nc.tensor.matmul(out=pscr[:, half + j2 * TC: half + (j2 + 1) * TC],
                                             lhsT=ktd[64:128, kt // 2, (kt % 2) * 128: (kt % 2 + 1) * 128],
                                             rhs=qd[64:128, :],
                                             start=True, stop=True, tile_position=(64, 0))
                        pt = ptp.tile([128, 2 * half], BF16, tag=f"pt{bw}", bufs=2)
                        nc.scalar.activation(out=pt[:], in_=pscr[:],
                                             func=mybir.ActivationFunctionType.Exp, scale=0.125)
                        if PROBE == "exp2x":
                            ptx = ptp.tile([128, 2 * half], BF16, tag=f"ptx{bw}", bufs=2)
                            nc.scalar.activation(out=ptx[:], in_=pscr[:],
                                                 func=mybir.ActivationFunctionType.Exp, scale=0.125)
                        for j2 in range(bw):
                            kt = kt0 + j2
                            if PROBE == "mm22x":
                                poX = pso.tile([65, TC], F32, tag="poX", bufs=2)
                                nc.tensor.matmul(out=poX[:], lhsT=vfull[:, kt, hA, :],
                                                 rhs=pt[:, j2 * TC: (j2 + 1) * TC],
                                                 start=True, stop=True)
                                nc.tensor.matmul(out=poX[:], lhsT=vfull[:, kt, hA + 1, :],
                                                 rhs=pt[:, half + j2 * TC: half + (j2 + 1) * TC],
                                                 start=False, stop=True, skip_group_check=True)
                            nc.tensor.matmul(out=poA[:], lhsT=vfull[:, kt, hA, :],
                                             rhs=pt[:, j2 * TC: (j2 + 1) * TC],
                                             start=(kt == 0), stop=(kt == 15))
                            nc.tensor.matmul(out=poB[:], lhsT=vfull[:, kt, hA + 1, :],
                                             rhs=pt[:, half + j2 * TC: half + (j2 + 1) * TC],
                                             start=(kt == 0), stop=(kt == 15))
                        kt0 += bw

                    # evacuate O' to SBUF promptly so the PSUM banks free up
                    # for the next duo (the normalize chain below has a DRAM
                    # round-trip we must keep off the PE critical path)
                    if EVAC:
                        oA = nrm.tile([65, TC], F32, tag="oA")
                        nc.vector.tensor_copy(out=oA[:], in_=poA[:])
                        oB = nrm.tile([65, TC], F32, tag="oB")
                        nc.vector.tensor_copy(out=oB[:], in_=poB[:])
                    else:
                        oA, oB = poA, poB

                    # normalize: rec = 1/denominator, broadcast via DRAM bounce
                    recA = nrm.tile([1, TC], F32, tag="recA")
                    nc.vector.reciprocal(out=recA[:], in_=oA[64:65, :])
                    recB = nrm.tile([1, TC], F32, tag="recB")
                    nc.vector.reciprocal(out=recB[:], in_=oB[64:65, :])
                    rec_d = dram_sc.tile([2, TC], F32, tag="rec_d")
                    nc.sync.dma_start(out=rec_d[0:1, :], in_=recA[:])
                    nc.sync.dma_start(out=rec_d[1:2, :], in_=recB[:])
                    rbA = nrm.tile([64, TC], F32, tag="rbA")
                    nc.gpsimd.dma_start(out=rbA[:], in_=bcast_rows(rec_d[0:1, :], 64))
                    rbB = nrm.tile([64, TC], F32, tag="rbB")
                    nc.gpsimd.dma_start(out=rbB[:], in_=bcast_rows(rec_d[1:2, :], 64))
                    jt, rhalf = d // 2, (d % 2) * 64
                    nc.vector.tensor_tensor(out=attnv[0][rhalf: rhalf + 64, jt, :],
                                            in0=oA[0:64, :], in1=rbA[:],
                                            op=mybir.AluOpType.mult)
                    nc.vector.tensor_tensor(out=attnv[1][rhalf: rhalf + 64, jt, :],
                                            in0=oB[0:64, :], in1=rbB[:],
                                            op=mybir.AluOpType.mult)

            # ---------------- phase 4: o_net + residual + layernorm ----
            with tc.tile_pool(name="fin", bufs=2) as fin, \
                 tc.tile_pool(name="finc", bufs=1) as finc, \
                 tc.tile_pool(name="psf", bufs=4, space="PSUM") as psf:
                wo_sb = finc.tile([128, 8, 1024], BF16)
                nc.sync.dma_start(out=wo_sb[:], in_=woT[:])
                res_sb = finc.tile([128, 4, D], F32)
                nc.sync.dma_start(out=res_sb[:],
                                  in_=inp_res.rearrange("(c p) d -> p c d", p=128))
                gb_sb = finc.tile([128, D], F32)
                nc.gpsimd.dma_start(out=gb_sb[:], in_=bcast_rows(gamma[0:1, :], 128))
                bb_sb = finc.tile([128, D], F32)
                nc.gpsimd.dma_start(out=bb_sb[:], in_=bcast_rows(beta[0:1, :], 128))
                eps_sb = finc.tile([128, 1], F32)
                nc.vector.memset(eps_sb[:], LN_EPS)

                for b2 in range(2):
                    for tch in range(2):
                        chunk = b2 * 2 + tch
                        x = fin.tile([128, D], F32, tag="x")
                        for nn_ in range(2):
                            po = psf.tile([128, 512], F32, tag="po")
                            for jt in range(8):
                                nc.tensor.matmul(out=po[:],
                                                 lhsT=attnv[b2][:, jt, tch * 128: (tch + 1) * 128],
                                                 rhs=wo_sb[:, jt, nn_ * 512: (nn_ + 1) * 512],
                                                 start=(jt == 0), stop=(jt == 7))
                            nc.vector.tensor_tensor(out=x[:, nn_ * 512: (nn_ + 1) * 512],
                                                    in0=po[:],
                                                    in1=res_sb[:, chunk, nn_ * 512: (nn_ + 1) * 512],
                                                    op=mybir.AluOpType.add)
                        stats = fin.tile([128, 2, 6], F32, tag="stats")
                        for s2 in range(2):
                            nc.vector.bn_stats(out=stats[:, s2, :], in_=x[:, s2 * 512: (s2 + 1) * 512])
                        mv = fin.tile([128, 2], F32, tag="mv")
                        nc.vector.bn_aggr(out=mv[:], in_=stats[:])
                        sd = fin.tile([128, 1], F32, tag="sd")
                        nc.scalar.activation(out=sd[:], in_=mv[:, 1:2],
                                             func=mybir.ActivationFunctionType.Sqrt,
                                             bias=eps_sb[:], scale=1.0)
                        rstd = fin.tile([128, 1], F32, tag="rstd")
                        nc.vector.reciprocal(out=rstd[:], in_=sd[:])
                        y = fin.tile([128, D], F32, tag="y")
                        nc.vector.tensor_scalar(out=y[:], in0=x[:],
                                                scalar1=mv[:, 0:1], scalar2=rstd[:],
                                                op0=mybir.AluOpType.subtract,
                                                op1=mybir.AluOpType.mult)
                        yg = fin.tile([128, D], F32, tag="yg")
                        nc.vector.tensor_tensor(out=yg[:], in0=y[:], in1=gb_sb[:],
                                                op=mybir.AluOpType.mult)
                        yb = fin.tile([128, D], F32, tag="yb")
                        nc.vector.tensor_tensor(out=yb[:], in0=yg[:], in1=bb_sb[:],
                                                op=mybir.AluOpType.add)
                        nc.sync.dma_start(
                            out=out[chunk * 128: (chunk + 1) * 128, :], in_=yb[:])

    nc.finalize()
    return nc


def _get_program(reps=1, score_bf16=None):
    import os as _os
    if score_bf16 is None:
        score_bf16 = SCORE_BF16
    key = ("nc", reps, score_bf16, _os.environ.get("KBLOCKS", ""), _os.environ.get("KPSB", ""), _os.environ.get("KEVAC", ""), _os.environ.get("KPROBE", ""))
    if key not in _prog_cache:
        _prog_cache[key] = _build_program(reps, score_bf16)
    return _prog_cache[key]


def _prep_inputs(inp, W_qkv, b_qkv, W_o, gamma, beta):
    """Build the 8 per-core input dicts (host-side, all free)."""
    f32 = np.float32
    inp = np.asarray(inp, f32)
    W_qkv = np.asarray(W_qkv, f32)
    b_qkv = np.asarray(b_qkv, f32)
    W_o = np.asarray(W_o, f32)
    gamma = np.asarray(gamma, f32).reshape(1, D)
    beta = np.asarray(beta, f32).reshape(1, D)

    wqkT = np.ascontiguousarray(
        W_qkv[0:2048, :].T.reshape(8, 128, 2048).transpose(1, 0, 2)).astype(nbf16)
    wvT = np.ascontiguousarray(
        W_qkv[2048:3072, :].T.reshape(8, 128, 1024).transpose(1, 0, 2)).astype(nbf16)
    woT = np.ascontiguousarray(
        W_o.T.reshape(8, 128, 1024).transpose(1, 0, 2)).astype(nbf16)
    b_qk = b_qkv[0:2048].reshape(1, 2048).astype(nbf16)
    b_v = b_qkv[2048:3072].reshape(1, 1024).astype(nbf16)
    ones = np.ones((1, NTOK), nbf16)

    in_maps = []
    for c in range(N_CORES):
        sl = slice(c * TC, (c + 1) * TC)
        x = np.concatenate([inp[0, sl, :], inp[1, sl, :]], axis=0)  # [512, 1024]
        inpT = np.ascontiguousarray(
            x.T.reshape(8, 128, NTOK).transpose(1, 0, 2)).astype(nbf16)
        in_maps.append({
            "inpT": inpT,
            "inp_res": np.ascontiguousarray(x),
            "wqkT": wqkT, "wvT": wvT, "woT": woT,
            "b_qk": b_qk, "b_v": b_v, "onesd": ones,
            "gamma": gamma, "beta": beta,
        })
    return in_maps


def _assemble(results):
    out = np.empty((B, T, D), np.float32)
    for c in range(N_CORES):
        o = results[c]["out"]
        sl = slice(c * TC, (c + 1) * TC)
        out[0, sl, :] = o[0:TC, :]
        out[1, sl, :] = o[TC:NTOK, :]
    return out


def kernel(inp, W_qkv, b_qkv, W_o, gamma, beta):
    nc = _get_program()
    in_maps = _prep_inputs(inp, W_qkv, b_qkv, W_o, gamma, beta)
    res = run_bass_kernel_spmd(nc, in_maps, core_ids=list(range(N_CORES)))
    return _assemble(res.results)


if __name__ == "__main__":
    rng = np.random.RandomState(0)
    inp = rng.randn(B, T, D).astype(np.float32)
    W_qkv = (rng.randn(3 * H * DH, D) * D ** -0.5).astype(np.float32)
    b_qkv = (rng.randn(3 * H * DH) * 0.02).astype(np.float32)
    W_o = (rng.randn(D, H * DH) * (H * DH) ** -0.5).astype(np.float32)
    gamma = np.ones(D, np.float32)
    beta = np.zeros(D, np.float32)
    out = kernel(inp=inp, W_qkv=W_qkv, b_qkv=b_qkv, W_o=W_o, gamma=gamma, beta=beta)
    print("out", out.shape, out.dtype, np.abs(out).mean())
